# revision 1
# baseline (speedup 1.0000x reference)
"""Trainium2 Bass kernel for a GRU encoder-decoder (KLCPD generator).

Model (see reference):
  past_emb = relu(past @ W_emb + b_emb)            [T,B,E]
  fut_emb  = relu(future @ W_emb + b_emb)          [T,B,E]
  _, h_T   = GRU_enc(past_emb, h0=0)
  hidden   = h_T + noise
  ys, _    = GRU_dec(shift(fut_emb), h0=hidden)
  out      = ys @ W_out + b_out                    [T,B,D]

Sharding: data-parallel over batch B=1024 across 8 NeuronCores
(B_local=128); all weights replicated; no collectives.

Per-core kernel layout decisions:
  * All matmul inputs are bf16 (fp32 accumulation in PSUM).
  * The GRU hidden state is kept *transposed* in SBUF as
    hT[p, k*128 + b] = h[b, k*128 + p]  (k = H-chunk 0..3), so the
    elementwise gate math produces, with zero extra transposes, exactly
    the stationary operand needed by the next step's matmuls.
  * Gate pre-activations are accumulated in four PSUM banks (r, z, hn,
    xn) in the same transposed layout; the input contribution
    x_emb @ W_ih of step t is accumulated into the same banks before the
    recurrent matmuls so it runs on the PE while step t-1's gate tail is
    still executing on ACT/DVE.
  * Embeddings for both inputs are precomputed once (PE transposes of
    the [128,128] input tiles + matmul + relu) into SBUF-resident
    transposed bf16 tiles embT[e][128, T*128].
"""

import os
from contextlib import ExitStack

import numpy as np

import concourse.bass as bass
import concourse.tile as tile
from concourse import bacc, bass_utils, masks, mybir
from concourse.tile_rust import add_dep_helper

T, B, D, E, H = 64, 1024, 128, 256, 512
NCORES = 8
BL = B // NCORES  # 128
H3 = 3 * H
P = 128

f32 = mybir.dt.float32
bf16 = mybir.dt.bfloat16
AF = mybir.ActivationFunctionType
OP = mybir.AluOpType


def _mm(nc, out, lhsT, rhs, start, stop):
    nc.tensor.matmul(out, lhsT, rhs, start=start, stop=stop, skip_group_check=True)


# Tunables (swept via TimelineSim, validated on HW).
CFG = {
    "tail_halves": 2,     # 1 = full-width gate ops, 2 = H-halved
    "w_on_gpsimd": False,  # offload w = z*h to the Pool engine
    "force_order": False,  # order half-1 DVE ops after h'-half0
}


def build_module(zero_bias: bool, t_steps: int = T, dump_h: bool = False):
    """Builds the per-core Bass module. Returns the compiled nc."""
    nc = bacc.Bacc("TRN2", target_bir_lowering=False, debug=False)
    dbg_h = None
    if dump_h:
        dbg_h = nc.dram_tensor("dbg_h", [2, t_steps, P, H], bf16, kind="ExternalOutput").ap()

    past = nc.dram_tensor("past", [t_steps, BL, D], f32, kind="ExternalInput").ap()
    fut = nc.dram_tensor("fut", [t_steps, BL, D], f32, kind="ExternalInput").ap()
    noise = nc.dram_tensor("noise", [BL, H], f32, kind="ExternalInput").ap()
    w_emb = nc.dram_tensor("w_emb", [D, E], f32, kind="ExternalInput").ap()
    b_emb = nc.dram_tensor("b_emb", [1, E], f32, kind="ExternalInput").ap()
    wd = {}
    for g in ("enc", "dec"):
        wd[g, "ih"] = nc.dram_tensor(f"w_ih_{g}", [E, H3], f32, kind="ExternalInput").ap()
        wd[g, "hh"] = nc.dram_tensor(f"w_hh_{g}", [H, H3], f32, kind="ExternalInput").ap()
        wd[g, "bih"] = nc.dram_tensor(f"b_ih_{g}", [1, H3], f32, kind="ExternalInput").ap()
        wd[g, "bhh"] = nc.dram_tensor(f"b_hh_{g}", [1, H3], f32, kind="ExternalInput").ap()
    w_out = nc.dram_tensor("w_out", [H, D], f32, kind="ExternalInput").ap()
    b_out = nc.dram_tensor("b_out", [1, D], f32, kind="ExternalInput").ap()
    out = nc.dram_tensor("out", [t_steps, BL, D], f32, kind="ExternalOutput").ap()

    with tile.TileContext(nc, pool_alloc_mode="queue") as tc, ExitStack() as octx:
        wpool = octx.enter_context(tc.tile_pool(name="weights", bufs=1))

        # ---- constants -------------------------------------------------
        ident = wpool.tile([P, P], bf16)
        masks.make_identity(nc, ident[:])
        ones_row = wpool.tile([1, 512], bf16)
        nc.gpsimd.memset(ones_row[:], 1.0)

        # ---- weight preload + cast to bf16 -----------------------------
        whh = {}   # whh[g][k]: [128, H3]
        wih = {}   # wih[g][e]: [128, H3]
        biasx = {}  # [1, H3]  (b_ih + b_hh on r,z cols; b_ih on n cols)
        biashn = {}  # [1, 512] (b_hh n-part)
        with tc.tile_pool(name="wstage", bufs=2) as stage:
            wemb_bf = wpool.tile([P, E], bf16)
            st = stage.tile([P, E], f32, tag="s_emb")
            nc.sync.dma_start(st[:], w_emb[:, :])
            nc.vector.tensor_copy(wemb_bf[:], st[:])

            wout_bf = wpool.tile([P, 4 * P], bf16)  # col block k = W_out rows k
            st = stage.tile([P, 4 * P], f32, tag="s_out")
            for k in range(4):
                nc.sync.dma_start(st[:, k * P:(k + 1) * P], w_out[k * P:(k + 1) * P, :])
            nc.vector.tensor_copy(wout_bf[:], st[:])

            for g in ("enc", "dec"):
                wih[g] = []
                for e in range(2):
                    t_ = wpool.tile([P, H3], bf16, tag=f"wih_{g}_{e}")
                    st = stage.tile([P, H3], f32, tag="s_ih")
                    nc.sync.dma_start(st[:], wd[g, "ih"][e * P:(e + 1) * P, :])
                    nc.vector.tensor_copy(t_[:], st[:])
                    wih[g].append(t_)
                whh[g] = []
                for k in range(4):
                    t_ = wpool.tile([P, H3], bf16, tag=f"whh_{g}_{k}")
                    st = stage.tile([P, H3], f32, tag="s_hh")
                    nc.sync.dma_start(st[:], wd[g, "hh"][k * P:(k + 1) * P, :])
                    nc.scalar.copy(t_[:], st[:])
                    whh[g].append(t_)
                if not zero_bias:
                    sih = stage.tile([1, H3], f32, tag="s_bih")
                    shh = stage.tile([1, H3], f32, tag="s_bhh")
                    nc.sync.dma_start(sih[:], wd[g, "bih"][:, :])
                    nc.sync.dma_start(shh[:], wd[g, "bhh"][:, :])
                    bx = wpool.tile([1, H3], bf16, tag=f"biasx_{g}")
                    nc.vector.tensor_add(bx[:, 0:2 * H], sih[:, 0:2 * H], shh[:, 0:2 * H])
                    nc.vector.tensor_copy(bx[:, 2 * H:H3], sih[:, 2 * H:H3])
                    bh = wpool.tile([1, H], bf16, tag=f"biashn_{g}")
                    nc.vector.tensor_copy(bh[:], shh[:, 2 * H:H3])
                    biasx[g] = bx
                    biashn[g] = bh

            bemb_bf = None
            if not zero_bias:
                st = stage.tile([1, E], f32, tag="s_bemb")
                nc.sync.dma_start(st[:], b_emb[:, :])
                bemb_bf = wpool.tile([1, E], bf16)
                nc.vector.tensor_copy(bemb_bf[:], st[:])
                bout_bf = wpool.tile([1, D], bf16)
                st = stage.tile([1, D], f32, tag="s_bout")
                nc.sync.dma_start(st[:], b_out[:, :])
                nc.vector.tensor_copy(bout_bf[:], st[:])

            # noise, transposed: noiseT[p, k*128+b] = noise[b, k*128+p]
            noiseT = wpool.tile([P, H], bf16)
            st = stage.tile([P, H], f32, tag="s_noise")
            nc.sync.dma_start(st[:], noise[:, :])
            nbf = stage.tile([P, H], bf16, tag="s_noise_bf")
            nc.vector.tensor_copy(nbf[:], st[:])
            with tc.tile_pool(name="psum_noise", bufs=1, space="PSUM") as pn:
                pt = pn.tile([P, H], bf16)
                for k in range(4):
                    nc.tensor.transpose(pt[:, k * P:(k + 1) * P], nbf[:, k * P:(k + 1) * P], ident[:])
                nc.scalar.copy(noiseT[:], pt[:])

        # ---- embedding precompute --------------------------------------
        # embT[g][e][p, t*BL + b] = relu(x[t] @ W_emb + b_emb)[b, e*128+p]
        embT = {g: [wpool.tile([P, t_steps * BL], bf16, name=f"embT_{g}_{e}", tag=f"embT_{g}_{e}")
                    for e in range(2)]
                for g in ("enc", "dec")}
        n_grp = t_steps // 4
        with tc.tile_pool(name="estage", bufs=3) as ep, \
             tc.tile_pool(name="psum_emb", bufs=2, space="PSUM") as pep:
            for g, x_ap in (("enc", past), ("dec", fut)):
                for gi in range(n_grp):
                    xs = ep.tile([P, 4 * P], f32, tag="xs")
                    nc.sync.dma_start(
                        xs[:].rearrange("p (i d) -> p i d", i=4),
                        x_ap[4 * gi:4 * gi + 4].transpose([1, 0, 2]),
                    )
                    xbf = ep.tile([P, 4 * P], bf16, tag="xbf")
                    nc.vector.tensor_copy(xbf[:], xs[:])
                    ptr = pep.tile([P, 4 * P], bf16, tag="ptr")
                    for i in range(4):
                        nc.tensor.transpose(ptr[:, i * P:(i + 1) * P], xbf[:, i * P:(i + 1) * P], ident[:])
                    xT = ep.tile([P, 4 * P], bf16, tag="xT")
                    nc.scalar.copy(xT[:], ptr[:])
                    for e in range(2):
                        pe_ = pep.tile([P, 4 * P], f32, tag=f"pe{e}")
                        _mm(nc, pe_[:], wemb_bf[:, e * P:(e + 1) * P], xT[:],
                            start=True, stop=zero_bias)
                        if not zero_bias:
                            _mm(nc, pe_[:], bemb_bf[0:1, e * P:(e + 1) * P], ones_row[0:1, :],
                                start=False, stop=True)
                        dst = embT[g][e][:, gi * 4 * P:(gi + 1) * 4 * P]
                        if e == 0:
                            nc.scalar.activation(dst, pe_[:], AF.Relu)
                        else:
                            nc.vector.tensor_scalar_max(dst, pe_[:], 0.0)

        # Decoder hidden states round-trip through DRAM; the output
        # projection runs as a separate phase after the decoder loop so the
        # loop keeps all 8 PSUM banks for double-buffered gate accumulators.
        ysT = nc.dram_tensor("ysT", [t_steps, P, H], bf16, kind="Internal").ap()

        # ---- GRU loops --------------------------------------------------
        def gru_loop(g, is_dec, hT0, sb, pg):
            """Runs t_steps of GRU g. hT0 = initial transposed state (or None).
            Returns final hT tile."""
            hT_prev = hT0

            def emit_xw(t):
                """Allocate step t's PSUM banks and emit its input-projection
                matmuls. Returns the bank state for the recurrent matmuls and
                gate tail."""
                have_x = (not is_dec) or t > 0
                have_h = t > 0 or hT0 is not None
                have_xn = have_x or not zero_bias
                pr = pg.tile([P, H], f32, name="pr", tag="pr")
                pz = pg.tile([P, H], f32, name="pz", tag="pz")
                pxn = pg.tile([P, H], f32, name="pxn", tag="pxn") if have_xn else None
                phn = pg.tile([P, H], f32, name="phn", tag="phn") if have_h else None

                # One start=True per PSUM bank per step (the hardware's
                # pending-zero covers the whole 2KB bank); one stop on the
                # bank's last matmul. Track per-bank emitted/total counts.
                nbias = 0 if zero_bias else 1
                nxw = (2 if have_x else 0) + nbias
                nhw = 4 if have_h else 0
                totals = {id(pr): 4 * (nxw + nhw), id(pz): 4 * (nxw + nhw)}
                if pxn is not None:
                    totals[id(pxn)] = 4 * nxw
                if phn is not None:
                    totals[id(phn)] = 4 * (4 + nbias)
                emitted = {k: 0 for k in totals}

                def emit(bank, sl, lhsT, rhs):
                    emitted[id(bank)] += 1
                    _mm(nc, sl, lhsT, rhs,
                        start=emitted[id(bank)] == 1,
                        stop=emitted[id(bank)] == totals[id(bank)])

                tcol = (t - 1) if is_dec else t
                lx = ([embT[g][e][:, tcol * BL:(tcol + 1) * BL] for e in range(2)]
                      if have_x else None)
                for bank, lo in ((pr, 0), (pz, H), (pxn, 2 * H)):
                    if bank is None:
                        continue
                    for m in range(4):
                        sl = bank[:, m * P:(m + 1) * P]
                        if not zero_bias:
                            emit(bank, sl, biasx[g][0:1, lo + m * P:lo + (m + 1) * P],
                                 ones_row[0:1, 0:P])
                        if have_x:
                            for e in range(2):
                                emit(bank, sl, wih[g][e][:, lo + m * P:lo + (m + 1) * P], lx[e])
                return pr, pz, pxn, phn, emit

            state = emit_xw(0)
            for t in range(t_steps):
                have_x = (not is_dec) or t > 0
                have_h = hT_prev is not None
                have_xn = have_x or not zero_bias
                pr, pz, pxn, phn, emit = state

                # -- hW(t): recurrent matmuls. Batch all k∈{0,1} first (they
                # depend only on the first half of h'(t-1), which the tail
                # produces early), then k∈{2,3}. Step t+1's input-projection
                # matmuls are emitted BETWEEN the batches so the PE fills the
                # wait for h'(t-1)'s second half with useful work. ---------
                def hw_batch(kpair):
                    # bank-major: all r slices first (both gate halves), then
                    # hn, then z — the chain-critical sigmoid(r) of each half
                    # unblocks as early as possible.
                    for bank, lo in ((pr, 0), (phn, 2 * H), (pz, H)):
                        for m in range(4):
                            sl = bank[:, m * P:(m + 1) * P]
                            if bank is phn and not zero_bias and kpair[0] == 0:
                                emit(bank, sl, biashn[g][0:1, m * P:(m + 1) * P],
                                     ones_row[0:1, 0:P])
                            for k in kpair:
                                emit(bank, sl, whh[g][k][:, lo + m * P:lo + (m + 1) * P],
                                     hT_prev[:, k * P:(k + 1) * P])

                if have_h:
                    hw_batch((0, 1))
                if t + 1 < t_steps:
                    state = emit_xw(t + 1)
                if have_h:
                    hw_batch((2, 3))

                # -- gate math (transposed layout), chunked in two halves --
                HH = H // 2
                r_t = sb.tile([P, H], bf16, name="r_t", tag="r")
                z_t = sb.tile([P, H], bf16, name="z_t", tag="z")
                n_t = sb.tile([P, H], bf16, tag="n")
                p_t = sb.tile([P, H], bf16, tag="p")
                h_new = sb.tile([P, H], bf16, tag="h")
                if have_h:
                    t1 = sb.tile([P, H], bf16, tag="t1")
                    t2 = sb.tile([P, H], bf16, name="t2", tag="t2") if have_xn else t1
                    w_t = sb.tile([P, H], bf16, tag="w")
                h0_done = None
                nhalves = CFG["tail_halves"]
                hw_ = H // nhalves
                for half in range(nhalves):
                    hs = slice(half * hw_, (half + 1) * hw_)
                    half_ops = []
                    nc.scalar.activation(r_t[:, hs], pr[:, hs], AF.Sigmoid)
                    nc.scalar.activation(z_t[:, hs], pz[:, hs], AF.Sigmoid)
                    if have_h:
                        half_ops.append(nc.vector.tensor_mul(t1[:, hs], r_t[:, hs], phn[:, hs]))
                        if have_xn:
                            half_ops.append(nc.vector.tensor_add(t2[:, hs], t1[:, hs], pxn[:, hs]))
                        n_src = t2
                    else:
                        n_src = pxn
                    nc.scalar.activation(n_t[:, hs], n_src[:, hs], AF.Tanh)
                    if have_h:
                        weng = nc.gpsimd if CFG["w_on_gpsimd"] else nc.vector
                        half_ops.append(weng.tensor_mul(w_t[:, hs], z_t[:, hs], hT_prev[:, hs]))
                        half_ops.append(nc.vector.scalar_tensor_tensor(
                            p_t[:, hs], z_t[:, hs], 1.0, n_t[:, hs], OP.subtract, OP.mult))
                        hdone = nc.vector.tensor_sub(h_new[:, hs], w_t[:, hs], p_t[:, hs])
                    else:
                        half_ops.append(nc.vector.scalar_tensor_tensor(
                            p_t[:, hs], z_t[:, hs], 1.0, n_t[:, hs], OP.subtract, OP.mult))
                        hdone = nc.vector.tensor_scalar_mul(h_new[:, hs], p_t[:, hs], -1.0)
                    if half == 0:
                        h0_done = hdone
                    elif CFG["force_order"]:
                        # Keep the DVE focused on finishing h'-half0 before it
                        # starts half-1 work: the next step's k01 matmuls are
                        # gated on half 0.
                        for op in half_ops:
                            add_dep_helper(h0_done.ins, op.ins, sync=False,
                                           reason="finish h half0 first")
                hT_prev = h_new
                if dbg_h is not None:
                    nc.sync.dma_start(dbg_h[1 if is_dec else 0, t], h_new[:])
                if is_dec:
                    nc.sync.dma_start(ysT[t], h_new[:])
            return hT_prev

        with tc.tile_pool(name="gru_sb", bufs=3) as sb, \
             tc.tile_pool(name="psum_g", bufs=2, space="PSUM") as pg:
            hT_enc = gru_loop("enc", False, None, sb, pg)
            hid = sb.tile([P, H], bf16, tag="h")
            nc.vector.tensor_add(hid[:], hT_enc[:], noiseT[:])
            gru_loop("dec", True, hid, sb, pg)

        # ---- output projection phase: out = ys @ W_out + b_out ----------
        with tc.tile_pool(name="ostage", bufs=3) as osb_pool, \
             tc.tile_pool(name="psum_po", bufs=2, space="PSUM") as po_pool, \
             tc.tile_pool(name="psum_potr", bufs=2, space="PSUM") as potr_pool:
            for w in range(t_steps // 4):
                po = po_pool.tile([P, 4 * P], f32, tag="po")
                nmm = 4 + (0 if zero_bias else 1)
                if not zero_bias:
                    _mm(nc, po[:], bout_bf[0:1, :], ones_row[0:1, :], start=True, stop=False)
                for k in range(4):
                    rk = osb_pool.tile([P, 4 * P], bf16, name="rk", tag=f"rk{k}")
                    nc.sync.dma_start(
                        rk[:].rearrange("p (i b) -> p i b", i=4),
                        ysT[4 * w:4 * w + 4, :, k * P:(k + 1) * P].transpose([1, 0, 2]),
                    )
                    _mm(nc, po[:], wout_bf[:, k * P:(k + 1) * P], rk[:],
                        start=zero_bias and k == 0, stop=k == 3)
                osb = osb_pool.tile([P, 4 * P], bf16, tag="osb")
                nc.scalar.copy(osb[:], po[:])
                potr = potr_pool.tile([P, 4 * P], bf16, tag="potr")
                for i in range(4):
                    nc.tensor.transpose(potr[:, i * P:(i + 1) * P], osb[:, i * P:(i + 1) * P],
                                        ident[:])
                outf = osb_pool.tile([P, 4 * P], f32, tag="outf")
                nc.vector.tensor_copy(outf[:], potr[:])
                nc.sync.dma_start(
                    out[4 * w:4 * w + 4].transpose([1, 0, 2]),
                    outf[:].rearrange("p (i d) -> p i d", i=4),
                )

    nc.compile()
    return nc


_CACHE = {}


def _get_module(zero_bias: bool):
    key = zero_bias
    if key not in _CACHE:
        _CACHE[key] = build_module(zero_bias)
    return _CACHE[key]


def kernel(past_input, future_input, noise,
           W_emb, b_emb,
           W_ih_enc, W_hh_enc, b_ih_enc, b_hh_enc,
           W_ih_dec, W_hh_dec, b_ih_dec, b_hh_dec,
           W_out, b_out):
    f = np.float32
    past_input = np.asarray(past_input, f)
    future_input = np.asarray(future_input, f)
    noise = np.asarray(noise, f)
    zero_bias = not any(
        np.any(np.asarray(b)) for b in (b_emb, b_ih_enc, b_hh_enc, b_ih_dec, b_hh_dec, b_out)
    )
    nc = _get_module(zero_bias)

    shared = {
        "w_emb": np.asarray(W_emb, f),
        "b_emb": np.asarray(b_emb, f).reshape(1, E),
        "w_ih_enc": np.asarray(W_ih_enc, f), "w_hh_enc": np.asarray(W_hh_enc, f),
        "b_ih_enc": np.asarray(b_ih_enc, f).reshape(1, H3),
        "b_hh_enc": np.asarray(b_hh_enc, f).reshape(1, H3),
        "w_ih_dec": np.asarray(W_ih_dec, f), "w_hh_dec": np.asarray(W_hh_dec, f),
        "b_ih_dec": np.asarray(b_ih_dec, f).reshape(1, H3),
        "b_hh_dec": np.asarray(b_hh_dec, f).reshape(1, H3),
        "w_out": np.asarray(W_out, f),
        "b_out": np.asarray(b_out, f).reshape(1, D),
    }
    in_maps = []
    for c in range(NCORES):
        sl = slice(c * BL, (c + 1) * BL)
        m = dict(shared)
        m["past"] = np.ascontiguousarray(past_input[:, sl, :])
        m["fut"] = np.ascontiguousarray(future_input[:, sl, :])
        m["noise"] = np.ascontiguousarray(noise[sl, :])
        in_maps.append(m)

    res = bass_utils.run_bass_kernel_spmd(nc, in_maps, core_ids=list(range(NCORES)))
    return np.concatenate([r["out"] for r in res.results], axis=1)



# revision 2
# speedup vs baseline: 2.1827x; 2.1827x over previous
"""Trainium2 Bass kernel for a GRU encoder-decoder (KLCPD generator).

Model (see reference):
  past_emb = relu(past @ W_emb + b_emb)            [T,B,E]
  fut_emb  = relu(future @ W_emb + b_emb)          [T,B,E]
  _, h_T   = GRU_enc(past_emb, h0=0)
  hidden   = h_T + noise
  ys, _    = GRU_dec(shift(fut_emb), h0=hidden)
  out      = ys @ W_out + b_out                    [T,B,D]

Sharding: data-parallel over batch B=1024 across 8 NeuronCores
(B_local=128); all weights replicated; no collectives.

Per-core kernel layout decisions:
  * All matmul inputs are bf16 (fp32 accumulation in PSUM).
  * The GRU hidden state is kept *transposed* in SBUF as
    hT[p, k*128 + b] = h[b, k*128 + p]  (k = H-chunk 0..3), so the
    elementwise gate math produces, with zero extra transposes, exactly
    the stationary operand needed by the next step's matmuls.
  * Gate pre-activations are accumulated in four PSUM banks (r, z, hn,
    xn) in the same transposed layout; the input contribution
    x_emb @ W_ih of step t is accumulated into the same banks before the
    recurrent matmuls so it runs on the PE while step t-1's gate tail is
    still executing on ACT/DVE.
  * Embeddings for both inputs are precomputed once (PE transposes of
    the [128,128] input tiles + matmul + relu) into SBUF-resident
    transposed bf16 tiles embT[e][128, T*128].
"""

import os
from contextlib import ExitStack

import numpy as np

import concourse.bass as bass
import concourse.tile as tile
from concourse import bacc, bass_utils, masks, mybir
from concourse.tile_rust import add_dep_helper

T, B, D, E, H = 64, 1024, 128, 256, 512
NCORES = 8
BL = B // NCORES  # 128
H3 = 3 * H
P = 128

f32 = mybir.dt.float32
bf16 = mybir.dt.bfloat16
AF = mybir.ActivationFunctionType
OP = mybir.AluOpType


def _mm(nc, out, lhsT, rhs, start, stop):
    nc.tensor.matmul(out, lhsT, rhs, start=start, stop=stop, skip_group_check=True)


# Tunables (swept via TimelineSim, validated on HW).
CFG = {
    "tail_halves": 2,     # 1 = full-width gate ops, 2 = H-halved
    "w_on_gpsimd": False,  # offload w = z*h to the Pool engine
    "force_order": False,  # order half-1 DVE ops after h'-half0
}


def build_module(zero_bias: bool, t_steps: int = T, dump_h: bool = False):
    """Builds the per-core Bass module. Returns the compiled nc."""
    nc = bacc.Bacc("TRN2", target_bir_lowering=False, debug=False)
    dbg_h = None
    if dump_h:
        dbg_h = nc.dram_tensor("dbg_h", [2, t_steps, P, H], bf16, kind="ExternalOutput").ap()

    past = nc.dram_tensor("past", [t_steps, BL, D], f32, kind="ExternalInput").ap()
    fut = nc.dram_tensor("fut", [t_steps, BL, D], f32, kind="ExternalInput").ap()
    noise = nc.dram_tensor("noise", [BL, H], f32, kind="ExternalInput").ap()
    w_emb = nc.dram_tensor("w_emb", [D, E], f32, kind="ExternalInput").ap()
    b_emb = nc.dram_tensor("b_emb", [1, E], f32, kind="ExternalInput").ap()
    wd = {}
    for g in ("enc", "dec"):
        wd[g, "ih"] = nc.dram_tensor(f"w_ih_{g}", [E, H3], f32, kind="ExternalInput").ap()
        wd[g, "hh"] = nc.dram_tensor(f"w_hh_{g}", [H, H3], f32, kind="ExternalInput").ap()
        wd[g, "bih"] = nc.dram_tensor(f"b_ih_{g}", [1, H3], f32, kind="ExternalInput").ap()
        wd[g, "bhh"] = nc.dram_tensor(f"b_hh_{g}", [1, H3], f32, kind="ExternalInput").ap()
    w_out = nc.dram_tensor("w_out", [H, D], f32, kind="ExternalInput").ap()
    b_out = nc.dram_tensor("b_out", [1, D], f32, kind="ExternalInput").ap()
    out = nc.dram_tensor("out", [t_steps, BL, D], f32, kind="ExternalOutput").ap()

    with tile.TileContext(nc, pool_alloc_mode="queue") as tc, ExitStack() as octx:
        wpool = octx.enter_context(tc.tile_pool(name="weights", bufs=1))

        # ---- constants -------------------------------------------------
        ident = wpool.tile([P, P], bf16)
        masks.make_identity(nc, ident[:])
        ones_row = wpool.tile([1, 512], bf16)
        nc.gpsimd.memset(ones_row[:], 1.0)

        # ---- weight preload + cast to bf16 -----------------------------
        whh = {}   # whh[g][k]: [128, H3]
        wih = {}   # wih[g][e]: [128, H3]
        biasx = {}  # [1, H3]  (b_ih + b_hh on r,z cols; b_ih on n cols)
        biashn = {}  # [1, 512] (b_hh n-part)
        with tc.tile_pool(name="wstage", bufs=2) as stage:
            wemb_bf = wpool.tile([P, E], bf16)
            st = stage.tile([P, E], f32, tag="s_emb")
            nc.sync.dma_start(st[:], w_emb[:, :])
            nc.vector.tensor_copy(wemb_bf[:], st[:])

            wout_bf = wpool.tile([P, 4 * P], bf16)  # col block k = W_out rows k
            st = stage.tile([P, 4 * P], f32, tag="s_out")
            for k in range(4):
                nc.sync.dma_start(st[:, k * P:(k + 1) * P], w_out[k * P:(k + 1) * P, :])
            nc.vector.tensor_copy(wout_bf[:], st[:])

            for g in ("enc", "dec"):
                wih[g] = []
                for e in range(2):
                    t_ = wpool.tile([P, H3], bf16, tag=f"wih_{g}_{e}")
                    st = stage.tile([P, H3], f32, tag="s_ih")
                    nc.sync.dma_start(st[:], wd[g, "ih"][e * P:(e + 1) * P, :])
                    nc.vector.tensor_copy(t_[:], st[:])
                    wih[g].append(t_)
                whh[g] = []
                for k in range(4):
                    t_ = wpool.tile([P, H3], bf16, tag=f"whh_{g}_{k}")
                    st = stage.tile([P, H3], f32, tag="s_hh")
                    nc.sync.dma_start(st[:], wd[g, "hh"][k * P:(k + 1) * P, :])
                    nc.scalar.copy(t_[:], st[:])
                    whh[g].append(t_)
                if not zero_bias:
                    sih = stage.tile([1, H3], f32, tag="s_bih")
                    shh = stage.tile([1, H3], f32, tag="s_bhh")
                    nc.sync.dma_start(sih[:], wd[g, "bih"][:, :])
                    nc.sync.dma_start(shh[:], wd[g, "bhh"][:, :])
                    bx = wpool.tile([1, H3], bf16, tag=f"biasx_{g}")
                    nc.vector.tensor_add(bx[:, 0:2 * H], sih[:, 0:2 * H], shh[:, 0:2 * H])
                    nc.vector.tensor_copy(bx[:, 2 * H:H3], sih[:, 2 * H:H3])
                    bh = wpool.tile([1, H], bf16, tag=f"biashn_{g}")
                    nc.vector.tensor_copy(bh[:], shh[:, 2 * H:H3])
                    biasx[g] = bx
                    biashn[g] = bh

            bemb_bf = None
            if not zero_bias:
                st = stage.tile([1, E], f32, tag="s_bemb")
                nc.sync.dma_start(st[:], b_emb[:, :])
                bemb_bf = wpool.tile([1, E], bf16)
                nc.vector.tensor_copy(bemb_bf[:], st[:])
                bout_bf = wpool.tile([1, D], bf16)
                st = stage.tile([1, D], f32, tag="s_bout")
                nc.sync.dma_start(st[:], b_out[:, :])
                nc.vector.tensor_copy(bout_bf[:], st[:])

            # noise, transposed: noiseT[p, k*128+b] = noise[b, k*128+p]
            noiseT = wpool.tile([P, H], bf16)
            st = stage.tile([P, H], f32, tag="s_noise")
            nc.sync.dma_start(st[:], noise[:, :])
            nbf = stage.tile([P, H], bf16, tag="s_noise_bf")
            nc.vector.tensor_copy(nbf[:], st[:])
            with tc.tile_pool(name="psum_noise", bufs=1, space="PSUM") as pn:
                pt = pn.tile([P, H], bf16)
                for k in range(4):
                    nc.tensor.transpose(pt[:, k * P:(k + 1) * P], nbf[:, k * P:(k + 1) * P], ident[:])
                nc.scalar.copy(noiseT[:], pt[:])

        # ---- embedding precompute --------------------------------------
        # embT[g][e][p, t*BL + b] = relu(x[t] @ W_emb + b_emb)[b, e*128+p]
        embT = {g: [wpool.tile([P, t_steps * BL], bf16, name=f"embT_{g}_{e}", tag=f"embT_{g}_{e}")
                    for e in range(2)]
                for g in ("enc", "dec")}
        n_grp = t_steps // 4
        with tc.tile_pool(name="estage", bufs=3) as ep, \
             tc.tile_pool(name="psum_emb", bufs=2, space="PSUM") as pep:
            for g, x_ap in (("enc", past), ("dec", fut)):
                for gi in range(n_grp):
                    xs = ep.tile([P, 4 * P], f32, tag="xs")
                    nc.sync.dma_start(
                        xs[:].rearrange("p (i d) -> p i d", i=4),
                        x_ap[4 * gi:4 * gi + 4].transpose([1, 0, 2]),
                    )
                    xbf = ep.tile([P, 4 * P], bf16, tag="xbf")
                    nc.vector.tensor_copy(xbf[:], xs[:])
                    ptr = pep.tile([P, 4 * P], bf16, tag="ptr")
                    for i in range(4):
                        nc.tensor.transpose(ptr[:, i * P:(i + 1) * P], xbf[:, i * P:(i + 1) * P], ident[:])
                    xT = ep.tile([P, 4 * P], bf16, tag="xT")
                    nc.scalar.copy(xT[:], ptr[:])
                    for e in range(2):
                        pe_ = pep.tile([P, 4 * P], f32, tag=f"pe{e}")
                        _mm(nc, pe_[:], wemb_bf[:, e * P:(e + 1) * P], xT[:],
                            start=True, stop=zero_bias)
                        if not zero_bias:
                            _mm(nc, pe_[:], bemb_bf[0:1, e * P:(e + 1) * P], ones_row[0:1, :],
                                start=False, stop=True)
                        dst = embT[g][e][:, gi * 4 * P:(gi + 1) * 4 * P]
                        if e == 0:
                            nc.scalar.activation(dst, pe_[:], AF.Relu)
                        else:
                            nc.vector.tensor_scalar_max(dst, pe_[:], 0.0)

        # Decoder hidden states round-trip through DRAM; the output
        # projection runs as a separate phase after the decoder loop so the
        # loop keeps all 8 PSUM banks for double-buffered gate accumulators.
        ysT = nc.dram_tensor("ysT", [t_steps, P, H], bf16, kind="Internal").ap()

        # ---- GRU loops --------------------------------------------------
        def gru_loop(g, is_dec, hT0, sb, pg):
            """Runs t_steps of GRU g. hT0 = initial transposed state (or None).
            Returns final hT tile."""
            hT_prev = hT0

            def emit_xw(t):
                """Allocate step t's PSUM banks and emit its input-projection
                matmuls. Returns the bank state for the recurrent matmuls and
                gate tail."""
                have_x = (not is_dec) or t > 0
                have_h = t > 0 or hT0 is not None
                have_xn = have_x or not zero_bias
                pr = pg.tile([P, H], f32, name="pr", tag="pr")
                pz = pg.tile([P, H], f32, name="pz", tag="pz")
                pxn = pg.tile([P, H], f32, name="pxn", tag="pxn") if have_xn else None
                phn = pg.tile([P, H], f32, name="phn", tag="phn") if have_h else None

                # One start=True per PSUM bank per step (the hardware's
                # pending-zero covers the whole 2KB bank); one stop on the
                # bank's last matmul. Track per-bank emitted/total counts.
                nbias = 0 if zero_bias else 1
                nxw = (2 if have_x else 0) + nbias
                nhw = 4 if have_h else 0
                totals = {id(pr): 4 * (nxw + nhw), id(pz): 4 * (nxw + nhw)}
                if pxn is not None:
                    totals[id(pxn)] = 4 * nxw
                if phn is not None:
                    totals[id(phn)] = 4 * (4 + nbias)
                emitted = {k: 0 for k in totals}

                def emit(bank, sl, lhsT, rhs):
                    emitted[id(bank)] += 1
                    _mm(nc, sl, lhsT, rhs,
                        start=emitted[id(bank)] == 1,
                        stop=emitted[id(bank)] == totals[id(bank)])

                tcol = (t - 1) if is_dec else t
                lx = ([embT[g][e][:, tcol * BL:(tcol + 1) * BL] for e in range(2)]
                      if have_x else None)
                for bank, lo in ((pr, 0), (pz, H), (pxn, 2 * H)):
                    if bank is None:
                        continue
                    for m in range(4):
                        sl = bank[:, m * P:(m + 1) * P]
                        if not zero_bias:
                            emit(bank, sl, biasx[g][0:1, lo + m * P:lo + (m + 1) * P],
                                 ones_row[0:1, 0:P])
                        if have_x:
                            for e in range(2):
                                emit(bank, sl, wih[g][e][:, lo + m * P:lo + (m + 1) * P], lx[e])
                return pr, pz, pxn, phn, emit

            state = emit_xw(0)
            for t in range(t_steps):
                have_x = (not is_dec) or t > 0
                have_h = hT_prev is not None
                have_xn = have_x or not zero_bias
                pr, pz, pxn, phn, emit = state

                # -- hW(t): recurrent matmuls. k∈{0,1} (gated on h'(t-1)
                # quarters 0/1, which the quartered tail produces early),
                # then k∈{2,3}. Step t+1's input-projection matmuls are
                # emitted AFTER the recurrent wave: they are dependency-free
                # and fill the PE wait for h'(t)'s first quarter. ----------
                def hw_batch(kpair):
                    # bank-major: all r slices first, then hn, then z — the
                    # chain-critical sigmoid(r) unblocks as early as possible.
                    for bank, lo in ((pr, 0), (phn, 2 * H), (pz, H)):
                        for m in range(4):
                            sl = bank[:, m * P:(m + 1) * P]
                            if bank is phn and not zero_bias and kpair[0] == 0:
                                emit(bank, sl, biashn[g][0:1, m * P:(m + 1) * P],
                                     ones_row[0:1, 0:P])
                            for k in kpair:
                                emit(bank, sl, whh[g][k][:, lo + m * P:lo + (m + 1) * P],
                                     hT_prev[:, k * P:(k + 1) * P])

                if have_h:
                    hw_batch((0, 1))
                    hw_batch((2, 3))
                if t + 1 < t_steps:
                    state = emit_xw(t + 1)

                # -- gate math (transposed layout). sigmoids run on H-halves
                # (better ACT amortization); the rest runs on 128-col
                # quarters so h'(t) quarter k unblocks the next step's
                # whh[k] matmuls as early as possible. ---------------------
                r_t = sb.tile([P, H], bf16, name="r_t", tag="r")
                z_t = sb.tile([P, H], bf16, name="z_t", tag="z")
                n_t = sb.tile([P, H], bf16, tag="n")
                p_t = sb.tile([P, H], bf16, tag="p")
                h_new = sb.tile([P, H], bf16, tag="h")
                if have_h:
                    t1 = sb.tile([P, H], bf16, tag="t1")
                    t2 = sb.tile([P, H], bf16, name="t2", tag="t2") if have_xn else t1
                    w_t = sb.tile([P, H], bf16, tag="w")

                def q(i):
                    return slice(i * P, (i + 1) * P)

                HH = H // 2
                for half in range(2):
                    hs = slice(half * HH, (half + 1) * HH)
                    nc.scalar.activation(r_t[:, hs], pr[:, hs], AF.Sigmoid)
                    nc.scalar.activation(z_t[:, hs], pz[:, hs], AF.Sigmoid)
                    if have_h:
                        # w = z * h(t-1) on the Pool engine — off the DVE.
                        nc.gpsimd.tensor_mul(w_t[:, hs], z_t[:, hs], hT_prev[:, hs])
                if have_h:
                    for i in range(4):
                        nc.vector.tensor_mul(t1[:, q(i)], r_t[:, q(i)], phn[:, q(i)])
                        if have_xn:
                            nc.vector.tensor_add(t2[:, q(i)], t1[:, q(i)], pxn[:, q(i)])
                        nc.scalar.activation(n_t[:, q(i)], t2[:, q(i)], AF.Tanh)
                        if i >= 1:
                            j = i - 1
                            nc.vector.scalar_tensor_tensor(
                                p_t[:, q(j)], z_t[:, q(j)], 1.0, n_t[:, q(j)],
                                OP.subtract, OP.mult)
                            nc.vector.tensor_sub(h_new[:, q(j)], w_t[:, q(j)], p_t[:, q(j)])
                    nc.vector.scalar_tensor_tensor(
                        p_t[:, q(3)], z_t[:, q(3)], 1.0, n_t[:, q(3)],
                        OP.subtract, OP.mult)
                    nc.vector.tensor_sub(h_new[:, q(3)], w_t[:, q(3)], p_t[:, q(3)])
                else:
                    for i in range(4):
                        nc.scalar.activation(n_t[:, q(i)], pxn[:, q(i)], AF.Tanh)
                        nc.vector.scalar_tensor_tensor(
                            p_t[:, q(i)], z_t[:, q(i)], 1.0, n_t[:, q(i)],
                            OP.subtract, OP.mult)
                        nc.vector.tensor_scalar_mul(h_new[:, q(i)], p_t[:, q(i)], -1.0)
                hT_prev = h_new
                if dbg_h is not None:
                    nc.sync.dma_start(dbg_h[1 if is_dec else 0, t], h_new[:])
                if is_dec:
                    nc.sync.dma_start(ysT[t], h_new[:])
            return hT_prev

        with tc.tile_pool(name="gru_sb", bufs=3) as sb, \
             tc.tile_pool(name="psum_g", bufs=2, space="PSUM") as pg:
            hT_enc = gru_loop("enc", False, None, sb, pg)
            hid = sb.tile([P, H], bf16, tag="h")
            nc.vector.tensor_add(hid[:], hT_enc[:], noiseT[:])
            gru_loop("dec", True, hid, sb, pg)

        # ---- output projection phase: out = ys @ W_out + b_out ----------
        with tc.tile_pool(name="ostage", bufs=3) as osb_pool, \
             tc.tile_pool(name="psum_po", bufs=2, space="PSUM") as po_pool, \
             tc.tile_pool(name="psum_potr", bufs=2, space="PSUM") as potr_pool:
            for w in range(t_steps // 4):
                po = po_pool.tile([P, 4 * P], f32, tag="po")
                nmm = 4 + (0 if zero_bias else 1)
                if not zero_bias:
                    _mm(nc, po[:], bout_bf[0:1, :], ones_row[0:1, :], start=True, stop=False)
                for k in range(4):
                    rk = osb_pool.tile([P, 4 * P], bf16, name="rk", tag=f"rk{k}")
                    nc.sync.dma_start(
                        rk[:].rearrange("p (i b) -> p i b", i=4),
                        ysT[4 * w:4 * w + 4, :, k * P:(k + 1) * P].transpose([1, 0, 2]),
                    )
                    _mm(nc, po[:], wout_bf[:, k * P:(k + 1) * P], rk[:],
                        start=zero_bias and k == 0, stop=k == 3)
                osb = osb_pool.tile([P, 4 * P], bf16, tag="osb")
                nc.scalar.copy(osb[:], po[:])
                potr = potr_pool.tile([P, 4 * P], bf16, tag="potr")
                for i in range(4):
                    nc.tensor.transpose(potr[:, i * P:(i + 1) * P], osb[:, i * P:(i + 1) * P],
                                        ident[:])
                outf = osb_pool.tile([P, 4 * P], f32, tag="outf")
                nc.vector.tensor_copy(outf[:], potr[:])
                nc.sync.dma_start(
                    out[4 * w:4 * w + 4].transpose([1, 0, 2]),
                    outf[:].rearrange("p (i d) -> p i d", i=4),
                )

    nc.compile()
    return nc


_CACHE = {}


def _get_module(zero_bias: bool):
    key = zero_bias
    if key not in _CACHE:
        _CACHE[key] = build_module(zero_bias)
    return _CACHE[key]


def kernel(past_input, future_input, noise,
           W_emb, b_emb,
           W_ih_enc, W_hh_enc, b_ih_enc, b_hh_enc,
           W_ih_dec, W_hh_dec, b_ih_dec, b_hh_dec,
           W_out, b_out):
    f = np.float32
    past_input = np.asarray(past_input, f)
    future_input = np.asarray(future_input, f)
    noise = np.asarray(noise, f)
    zero_bias = not any(
        np.any(np.asarray(b)) for b in (b_emb, b_ih_enc, b_hh_enc, b_ih_dec, b_hh_dec, b_out)
    )
    nc = _get_module(zero_bias)

    shared = {
        "w_emb": np.asarray(W_emb, f),
        "b_emb": np.asarray(b_emb, f).reshape(1, E),
        "w_ih_enc": np.asarray(W_ih_enc, f), "w_hh_enc": np.asarray(W_hh_enc, f),
        "b_ih_enc": np.asarray(b_ih_enc, f).reshape(1, H3),
        "b_hh_enc": np.asarray(b_hh_enc, f).reshape(1, H3),
        "w_ih_dec": np.asarray(W_ih_dec, f), "w_hh_dec": np.asarray(W_hh_dec, f),
        "b_ih_dec": np.asarray(b_ih_dec, f).reshape(1, H3),
        "b_hh_dec": np.asarray(b_hh_dec, f).reshape(1, H3),
        "w_out": np.asarray(W_out, f),
        "b_out": np.asarray(b_out, f).reshape(1, D),
    }
    in_maps = []
    for c in range(NCORES):
        sl = slice(c * BL, (c + 1) * BL)
        m = dict(shared)
        m["past"] = np.ascontiguousarray(past_input[:, sl, :])
        m["fut"] = np.ascontiguousarray(future_input[:, sl, :])
        m["noise"] = np.ascontiguousarray(noise[sl, :])
        in_maps.append(m)

    res = bass_utils.run_bass_kernel_spmd(nc, in_maps, core_ids=list(range(NCORES)))
    return np.concatenate([r["out"] for r in res.results], axis=1)

